# revision 15
# baseline (speedup 1.0000x reference)
"""Multi-head attention (B=2, L=4096, C=512, H=8, Dh=64) on 8 trn2 cores.

Sharding: data-parallel over batch (4 cores per batch element) x
tensor-parallel over heads (2 heads per core). Each core computes per-head
UNNORMALIZED partial outputs plus softmax denominators; the host divides by
the denominators, sums the partials, and adds the bias.

Per-core kernel (scores never hit HBM):
  - inputs (host pre-arranged for contiguous per-partition DMA lines):
    x[b].T as [p, c, n], wq/wk/wv col-slices as [p, c, m] with 1/sqrt(Dh)
    folded into wq, wo rows per head
  - Q^T [128, 4096] bf16; K^T stored zero-padded per head (kTz[:, h] has
    the other head's 64 rows zeroed) so the score matmuls run as K=128 in
    the same 128x128 PE tiling mode as everything else (a 64<->128 mode
    switch costs ~100ns/matmul in PE drain)
  - V [4096, 128] bf16 stored per head as [V_h(64) | ones(1) | pad(63)]
    so the AV matmul produces the softmax denominator in output row 64
  - attention in q-chunks of 512: per k-tile both heads' scores go into one
    [128, 1024] PSUM super-tile (2 banks, ring of 3). The exp alternates
    between engines per k-tile:
      even kt: ScalarE activation Exp (PSUM-direct -> bf16 SBUF)
      odd  kt: VectorE Schraudolph bit-trick exp: y_bits = int16(x*A + B)
               viewed as bf16 (exp2 linear-in-mantissa approximation; the
               softmax denominator cancels its common-mode error; end-to-end
               rel err ~6.5e-3 in simulation)
    This gives the PE two independent exp servers so the attention loop is
    PE-bound (the PE stays busy -> HAM keeps the 2.4GHz clock).
  - AV: per (kt, h) one bf16 matmul accumulating att^T [128,512] PSUM
    (1 bank per head) over the 32 k-tiles, lagging the scores by one k-tile
    so the PE FIFO never blocks on an exp.
  - att rows [0:65] (values + denominator) drain once per (qc, h) into
    per-head attn tiles; out-proj per head: out_h[q,:] = attn_h.T @ wo_h.
"""

import ml_dtypes
import numpy as np

B, L, C, H = 2, 4096, 512, 8
DH = C // H  # 64
P = 128
NCORES = 8
HEADS_PER_CORE = 2
CORES_PER_BATCH = 4

QCHUNK = 512  # q columns per attention block (1 PSUM bank per head)
NQC = L // QCHUNK  # 8
NKT = L // P  # 32 k-tiles
NCC = C // P  # 4 contraction chunks for projections

# Schraudolph bf16 exp: bits = int16(x * SCH_A + SCH_B); view as bf16
SCH_A = 128 * 1.4426950408889634  # 128 * log2(e)
SCH_B = 16248.5

_cached = {}


def _emit_av(nc, att, v_store, okt, opt):
    """AV for one k-tile, both heads, as K=128 matmuls (128x128 tiling mode,
    uniform with the zero-padded scores: a 64<->128 mode switch costs
    ~100ns/matmul in PE drain)."""
    for h in range(HEADS_PER_CORE):
        nc.tensor.matmul(
            att[h],
            v_store[:, okt, h, :],
            opt[:, h * QCHUNK:(h + 1) * QCHUNK],
            start=(okt == 0), stop=(okt == NKT - 1),
        )


def _build(reps=1):
    import concourse.mybir as mybir
    import concourse.tile as tile
    from concourse import bacc

    F32 = mybir.dt.float32
    BF16 = mybir.dt.bfloat16
    I16 = mybir.dt.int16
    EXP = mybir.ActivationFunctionType.Exp
    MULT = mybir.AluOpType.mult
    ADD = mybir.AluOpType.add

    nc = bacc.Bacc("TRN2", target_bir_lowering=False, debug=False,
                   num_devices=NCORES)
    xT = nc.dram_tensor("xT", [P, NCC, L], BF16, kind="ExternalInput").ap()
    # host pre-arranges W[:, cols] as [p, c, m] so the load is one
    # contiguous 1KB line per partition instead of 256B strided pieces
    wq = nc.dram_tensor("wq", [P, NCC, P], BF16, kind="ExternalInput").ap()
    wk = nc.dram_tensor("wk", [P, NCC, P], BF16, kind="ExternalInput").ap()
    wv = nc.dram_tensor("wv", [P, NCC, P], BF16, kind="ExternalInput").ap()
    wo = nc.dram_tensor("wo", [P, C], BF16, kind="ExternalInput").ap()
    # partition-major [p, qtile, c] so each store has 2KB-per-partition
    # contiguous lines (1KB lines halve DMA throughput); host re-layouts
    out0 = nc.dram_tensor("out0", [P, L // P, C], BF16,
                          kind="ExternalOutput").ap()
    out1 = nc.dram_tensor("out1", [P, L // P, C], BF16,
                          kind="ExternalOutput").ap()
    den01 = nc.dram_tensor("den01", [HEADS_PER_CORE, L], BF16,
                           kind="ExternalOutput").ap()

    with tile.TileContext(nc) as tc:
        import contextlib
        loop_cm = tc.For_i(0, reps, 1) if reps > 1 else contextlib.nullcontext()
        with (
            tc.tile_pool(name="persist", bufs=1) as persist,
            tc.tile_pool(name="xpool", bufs=1) as xpool,
            tc.tile_pool(name="ptp", bufs=6) as ptp,
            tc.tile_pool(name="small", bufs=2) as small,
            tc.tile_pool(name="outp", bufs=6) as outp,
            loop_cm,
        ):
            # preload the exp table set so the first real exp doesn't pay
            # the ~2.7us ACT_TABLE_LOAD mid-pipeline
            warm_in = small.tile([1, 8], F32, tag="warm_in")
            warm_out = small.tile([1, 8], F32, tag="warm_out")
            nc.vector.memset(warm_in, 0.0)
            nc.scalar.activation(warm_out, warm_in, EXP)

            # ---- load inputs ----
            wq_t = persist.tile([P, NCC, P], BF16)
            wk_t = persist.tile([P, NCC, P], BF16)
            wv_t = persist.tile([P, NCC, P], BF16)
            # wo rows per head, both at base partition 0 (matmul operands
            # must share base_partition with the attn lhsT)
            wo_th = [persist.tile([DH, C], BF16, name=f"wo_t{_h}")
                     for _h in range(HEADS_PER_CORE)]
            nc.sync.dma_start(wq_t, wq)
            nc.sync.dma_start(wk_t, wk)
            nc.sync.dma_start(wv_t, wv)
            for _h in range(HEADS_PER_CORE):
                nc.sync.dma_start(wo_th[_h], wo[_h * DH:(_h + 1) * DH, :])

            xt = xpool.tile([P, NCC, L], BF16)
            # x arrives host-pre-arranged as [p, c, n]: each DMA moves
            # contiguous 4KB-per-partition lines, split so the first
            # projection chunks can start before the full load lands
            for g in range(8):
                sl = slice(g * (L // 8), (g + 1) * (L // 8))
                for c in range(NCC):
                    nc.sync.dma_start(xt[:, c, sl], xT[:, c, sl])

            qT = persist.tile([P, L], BF16)
            # K^T zero-padded per head: kTz[:, h, :] has rows outside
            # [h*64, (h+1)*64) zeroed, so scores run as K=128 matmuls in the
            # same 128x128 tiling mode as everything else (mode switches
            # drain the PE, ~100ns/matmul)
            kTz = persist.tile([P, HEADS_PER_CORE, L], BF16)
            # per head block: [V_h (64) | ones (1) | zero pad (63)]
            v_store = persist.tile([P, NKT, HEADS_PER_CORE, P], BF16)
            # rows 0:64 = unnormalized attention, row 64 = denominator
            attn = [persist.tile([DH + 1, L], BF16, name=f"attn{_h}")
                    for _h in range(HEADS_PER_CORE)]

            # ---- projections ----
            with tc.tile_pool(name="pj_ps", bufs=2, space="PSUM") as pj_ps:
                nc.gpsimd.memset(kTz, 0.0)
                for j in range(L // 512):
                    sl = slice(j * 512, (j + 1) * 512)
                    ps = pj_ps.tile([P, 512], F32, tag="qk_ps")
                    for c in range(NCC):
                        nc.tensor.matmul(
                            ps, wk_t[:, c, :], xt[:, c, sl],
                            start=(c == 0), stop=(c == NCC - 1),
                        )
                    nc.scalar.copy(kTz[0:DH, 0, sl], ps[0:DH, :])
                    nc.vector.tensor_copy(kTz[DH:P, 1, sl], ps[DH:P, :])
                    ps = pj_ps.tile([P, 512], F32, tag="qk_ps")
                    for c in range(NCC):
                        nc.tensor.matmul(
                            ps, wq_t[:, c, :], xt[:, c, sl],
                            start=(c == 0), stop=(c == NCC - 1),
                        )
                    if j % 2 == 0:
                        nc.scalar.copy(qT[:, sl], ps)
                    else:
                        nc.vector.tensor_copy(qT[:, sl], ps)

                # V: per 128-token tile, [tokens, 128] = xT-chunk.T @ wv
                nc.gpsimd.memset(v_store, 0.0)
                ones_t = small.tile([P, NKT], F32, tag="ones")
                nc.vector.memset(ones_t, 1.0)
                for h in range(HEADS_PER_CORE):
                    nc.vector.tensor_copy(v_store[:, :, h, DH], ones_t)
                for r in range(NKT):
                    ps = pj_ps.tile([P, P], F32, tag="v_ps")
                    for c in range(NCC):
                        nc.tensor.matmul(
                            ps, xt[:, c, r * P:(r + 1) * P], wv_t[:, c, :],
                            start=(c == 0), stop=(c == NCC - 1),
                        )
                    nc.vector.tensor_copy(v_store[:, r, 0, 0:DH], ps[:, 0:DH])
                    nc.scalar.copy(v_store[:, r, 1, 0:DH], ps[:, DH:2 * DH])

            # ---- attention ----
            # sps ring 2 (4 banks) + att ring 4 (4 banks): the deeper att
            # ring lets AV of the next q-chunk start while the previous
            # chunk's drains are still queued behind exps on the engines;
            # the 2-deep score ring still covers the exp latency
            s_ps_cm = tc.tile_pool(name="s_ps", bufs=2, space="PSUM")
            a_ps_cm = tc.tile_pool(name="a_ps", bufs=4, space="PSUM")
            s_ps = s_ps_cm.__enter__()
            a_ps = a_ps_cm.__enter__()
            for qc in range(NQC):
                qsl = slice(qc * QCHUNK, (qc + 1) * QCHUNK)
                att = [a_ps.tile([P, QCHUNK], F32, tag="att", name=f"att{_h}")
                       for _h in range(HEADS_PER_CORE)]
                pend = []  # (kt, pt) waiting for their AV matmuls
                for kt in range(NKT):
                    sps = s_ps.tile([P, 2 * QCHUNK], F32, tag="spsum")
                    for h in range(HEADS_PER_CORE):
                        nc.tensor.matmul(
                            sps[:, h * QCHUNK:(h + 1) * QCHUNK],
                            kTz[:, h, kt * P:(kt + 1) * P],
                            qT[:, qsl],
                            start=True, stop=True,
                        )
                    pt = ptp.tile([P, 2 * QCHUNK], BF16, tag="pt")
                    if kt % 2 == 1:
                        nc.vector.tensor_scalar(
                            pt.bitcast(I16), sps, SCH_A, SCH_B, MULT, ADD)
                    else:
                        nc.scalar.activation(pt, sps, EXP)
                    pend.append((kt, pt))
                    if len(pend) > 1:
                        _emit_av(nc, att, v_store, *pend.pop(0))
                for p in pend:
                    _emit_av(nc, att, v_store, *p)
                # drain unnormalized attention + denominator row
                nc.scalar.copy(attn[0][:, qsl], att[0][0:DH + 1, :])
                nc.vector.tensor_copy(attn[1][:, qsl], att[1][0:DH + 1, :])
            a_ps_cm.__exit__(None, None, None)
            s_ps_cm.__exit__(None, None, None)

            # ---- output projection (per head, unnormalized) ----
            # all 8 PSUM banks are free here: deep ring keeps the PE dense
            for h in range(HEADS_PER_CORE):
                nc.sync.dma_start(den01[h:h + 1, :], attn[h][DH:DH + 1, :])
            with tc.tile_pool(name="o_ps", bufs=8, space="PSUM") as o_ps:
                osbs = [None, None]
                for qt in range(L // P):
                    for h, out_h in ((0, out0), (1, out1)):
                        ps = o_ps.tile([P, C], F32, tag="o_ps")
                        nc.tensor.matmul(
                            ps, attn[h][0:DH, qt * P:(qt + 1) * P],
                            wo_th[h], start=True, stop=True)
                        if qt % 4 == 0:
                            osbs[h] = outp.tile([P, 4, C], BF16, tag="osb",
                                                name=f"osb{h}")
                        if (2 * qt + h) % 2 == 0:
                            nc.scalar.copy(osbs[h][:, qt % 4, :], ps)
                        else:
                            nc.vector.tensor_copy(osbs[h][:, qt % 4, :], ps)
                        if qt % 4 == 3:
                            nc.sync.dma_start(
                                out_h[:, qt - 3:qt + 1, :], osbs[h])

    nc.compile()
    return nc


def _get_nc(reps=1):
    key = f"nc{reps}"
    if key not in _cached:
        _cached[key] = _build(reps)
    return _cached[key]


def _build_in_maps(inputs):
    x = np.asarray(inputs["x"], dtype=np.float32)
    Wq = np.asarray(inputs["Wq"], dtype=np.float32)
    Wk = np.asarray(inputs["Wk"], dtype=np.float32)
    Wv = np.asarray(inputs["Wv"], dtype=np.float32)
    Wo = np.asarray(inputs["Wo"], dtype=np.float32)

    scale = np.float32(1.0 / np.sqrt(DH))
    in_maps = []
    for core in range(NCORES):
        b = core // CORES_PER_BATCH
        j = core % CORES_PER_BATCH
        csl = slice(j * P, (j + 1) * P)
        bf = ml_dtypes.bfloat16
        in_maps.append({
            "xT": np.ascontiguousarray(
                x[b].T.reshape(NCC, P, L).transpose(1, 0, 2).astype(bf)),
            "wq": np.ascontiguousarray((Wq[:, csl] * scale).astype(bf)
                                       .reshape(NCC, P, P).transpose(1, 0, 2)),
            "wk": np.ascontiguousarray(Wk[:, csl].astype(bf)
                                       .reshape(NCC, P, P).transpose(1, 0, 2)),
            "wv": np.ascontiguousarray(Wv[:, csl].astype(bf)
                                       .reshape(NCC, P, P).transpose(1, 0, 2)),
            "wo": np.ascontiguousarray(Wo[csl, :].astype(bf)),
        })
    return in_maps


def kernel(x, Wq, Wk, Wv, Wo, bo):
    from concourse import bass_utils

    bo = np.asarray(bo, dtype=np.float32)
    in_maps = _build_in_maps(
        {"x": x, "Wq": Wq, "Wk": Wk, "Wv": Wv, "Wo": Wo})

    res = bass_utils.run_bass_kernel_spmd(
        _get_nc(), in_maps, core_ids=list(range(NCORES)))

    out = np.zeros((B, L, C), dtype=np.float32)
    for core in range(NCORES):
        r = res.results[core]
        den = np.asarray(r["den01"]).astype(np.float32)  # [2, L]
        b = core // CORES_PER_BATCH
        o0 = np.asarray(r["out0"]).astype(np.float32)
        o1 = np.asarray(r["out1"]).astype(np.float32)
        o0 = o0.transpose(1, 0, 2).reshape(L, C)
        o1 = o1.transpose(1, 0, 2).reshape(L, C)
        out[b] += o0 / den[0][:, None] + o1 / den[1][:, None]
    out += bo[None, None, :]
    return out


# revision 16
# speedup vs baseline: 1.2271x; 1.2271x over previous
"""Multi-head attention (B=2, L=4096, C=512, H=8, Dh=64) on 8 trn2 cores.

Sharding: data-parallel over batch (4 cores per batch element) x
tensor-parallel over heads (2 heads per core). Each core computes per-head
UNNORMALIZED partial outputs plus softmax denominators; the host divides by
the denominators, sums the partials, and adds the bias.

Per-core kernel (scores never hit HBM):
  - inputs (host pre-arranged for contiguous per-partition DMA lines):
    x[b].T as [p, c, n], wq/wk/wv col-slices as [p, c, m] with 1/sqrt(Dh)
    folded into wq, wo rows per head
  - Q^T [128, 4096] bf16; K^T stored zero-padded per head (kTz[:, h] has
    the other head's 64 rows zeroed) so the score matmuls run as K=128 in
    the same 128x128 PE tiling mode as everything else (a 64<->128 mode
    switch costs ~100ns/matmul in PE drain)
  - V [4096, 128] bf16 stored per head as [V_h(64) | ones(1) | pad(63)]
    so the AV matmul produces the softmax denominator in output row 64
  - attention in q-chunks of 512: per k-tile both heads' scores go into one
    [128, 1024] PSUM super-tile (2 banks, ring of 3). The exp alternates
    between engines per k-tile:
      even kt: ScalarE activation Exp (PSUM-direct -> bf16 SBUF)
      odd  kt: VectorE Schraudolph bit-trick exp: y_bits = int16(x*A + B)
               viewed as bf16 (exp2 linear-in-mantissa approximation; the
               softmax denominator cancels its common-mode error; end-to-end
               rel err ~6.5e-3 in simulation)
    This gives the PE two independent exp servers so the attention loop is
    PE-bound (the PE stays busy -> HAM keeps the 2.4GHz clock).
  - AV: per (kt, h) one bf16 matmul accumulating att^T [128,512] PSUM
    (1 bank per head) over the 32 k-tiles, lagging the scores by one k-tile
    so the PE FIFO never blocks on an exp.
  - att rows [0:65] (values + denominator) drain once per (qc, h) into
    per-head attn tiles; out-proj per head: out_h[q,:] = attn_h.T @ wo_h.
"""

import ml_dtypes
import numpy as np

B, L, C, H = 2, 4096, 512, 8
DH = C // H  # 64
P = 128
NCORES = 8
HEADS_PER_CORE = 2
CORES_PER_BATCH = 4

QCHUNK = 512  # q columns per attention block (1 PSUM bank per head)
NQC = L // QCHUNK  # 8
NKT = L // P  # 32 k-tiles
NCC = C // P  # 4 contraction chunks for projections

# Schraudolph bf16 exp: bits = int16(x * SCH_A + SCH_B); view as bf16
SCH_A = 128 * 1.4426950408889634  # 128 * log2(e)
SCH_B = 16248.5

_cached = {}


def _emit_av(nc, att, v_store, okt, opt):
    """AV for one k-tile, both heads, as K=128 matmuls (128x128 tiling mode,
    uniform with the zero-padded scores: a 64<->128 mode switch costs
    ~100ns/matmul in PE drain)."""
    for h in range(HEADS_PER_CORE):
        nc.tensor.matmul(
            att[h],
            v_store[:, okt, h, :],
            opt[:, h * QCHUNK:(h + 1) * QCHUNK],
            start=(okt == 0), stop=(okt == NKT - 1),
        )


def _build(reps=1):
    import concourse.mybir as mybir
    import concourse.tile as tile
    from concourse import bacc

    F32 = mybir.dt.float32
    BF16 = mybir.dt.bfloat16
    I16 = mybir.dt.int16
    EXP = mybir.ActivationFunctionType.Exp
    MULT = mybir.AluOpType.mult
    ADD = mybir.AluOpType.add

    nc = bacc.Bacc("TRN2", target_bir_lowering=False, debug=False,
                   num_devices=NCORES)
    xT = nc.dram_tensor("xT", [P, NCC, L], BF16, kind="ExternalInput").ap()
    # host pre-arranges W[:, cols] as [p, c, m] so the load is one
    # contiguous 1KB line per partition instead of 256B strided pieces
    wq = nc.dram_tensor("wq", [P, NCC, P], BF16, kind="ExternalInput").ap()
    wk = nc.dram_tensor("wk", [P, NCC, P], BF16, kind="ExternalInput").ap()
    wv = nc.dram_tensor("wv", [P, NCC, P], BF16, kind="ExternalInput").ap()
    wo = nc.dram_tensor("wo", [P, C], BF16, kind="ExternalInput").ap()
    # partition-major [p, qtile, c] so each store has 2KB-per-partition
    # contiguous lines (1KB lines halve DMA throughput); host re-layouts
    out0 = nc.dram_tensor("out0", [P, L // P, C], BF16,
                          kind="ExternalOutput").ap()
    out1 = nc.dram_tensor("out1", [P, L // P, C], BF16,
                          kind="ExternalOutput").ap()
    den01 = nc.dram_tensor("den01", [HEADS_PER_CORE, L], BF16,
                           kind="ExternalOutput").ap()

    with tile.TileContext(nc) as tc:
        import contextlib
        loop_cm = tc.For_i(0, reps, 1) if reps > 1 else contextlib.nullcontext()
        with (
            tc.tile_pool(name="persist", bufs=1) as persist,
            tc.tile_pool(name="xpool", bufs=1) as xpool,
            tc.tile_pool(name="ptp", bufs=6) as ptp,
            tc.tile_pool(name="small", bufs=2) as small,
            tc.tile_pool(name="outp", bufs=6) as outp,
            loop_cm,
        ):
            # preload the exp table set so the first real exp doesn't pay
            # the ~2.7us ACT_TABLE_LOAD mid-pipeline
            warm_in = small.tile([1, 8], F32, tag="warm_in")
            warm_out = small.tile([1, 8], F32, tag="warm_out")
            nc.vector.memset(warm_in, 0.0)
            nc.scalar.activation(warm_out, warm_in, EXP)

            # ---- load inputs ----
            wq_t = persist.tile([P, NCC, P], BF16)
            wk_t = persist.tile([P, NCC, P], BF16)
            wv_t = persist.tile([P, NCC, P], BF16)
            # wo rows per head, both at base partition 0 (matmul operands
            # must share base_partition with the attn lhsT)
            wo_th = [persist.tile([DH, C], BF16, name=f"wo_t{_h}")
                     for _h in range(HEADS_PER_CORE)]
            nc.sync.dma_start(wq_t, wq)
            nc.sync.dma_start(wk_t, wk)
            nc.sync.dma_start(wv_t, wv)
            for _h in range(HEADS_PER_CORE):
                nc.sync.dma_start(wo_th[_h], wo[_h * DH:(_h + 1) * DH, :])

            xt = xpool.tile([P, NCC, L], BF16)
            # x arrives host-pre-arranged as [p, c, n]: each DMA moves
            # contiguous 4KB-per-partition lines, split so the first
            # projection chunks can start before the full load lands
            for g in range(8):
                sl = slice(g * (L // 8), (g + 1) * (L // 8))
                for c in range(NCC):
                    nc.sync.dma_start(xt[:, c, sl], xT[:, c, sl])

            qT = persist.tile([P, L], BF16)
            # K^T zero-padded per head: kTz[:, h, :] has rows outside
            # [h*64, (h+1)*64) zeroed, so scores run as K=128 matmuls in the
            # same 128x128 tiling mode as everything else (mode switches
            # drain the PE, ~100ns/matmul)
            kTz = persist.tile([P, HEADS_PER_CORE, L], BF16)
            # per head block: [V_h (64) | ones (1) | zero pad (63)]
            v_store = persist.tile([P, NKT, HEADS_PER_CORE, P], BF16)
            # rows 0:64 = unnormalized attention, row 64 = denominator
            attn = [persist.tile([DH + 1, L], BF16, name=f"attn{_h}")
                    for _h in range(HEADS_PER_CORE)]

            # ---- projections ----
            with tc.tile_pool(name="pj_ps", bufs=2, space="PSUM") as pj_ps:
                nc.gpsimd.memset(kTz, 0.0)
                for j in range(L // 512):
                    sl = slice(j * 512, (j + 1) * 512)
                    ps = pj_ps.tile([P, 512], F32, tag="qk_ps")
                    for c in range(NCC):
                        nc.tensor.matmul(
                            ps, wk_t[:, c, :], xt[:, c, sl],
                            start=(c == 0), stop=(c == NCC - 1),
                        )
                    nc.scalar.copy(kTz[0:DH, 0, sl], ps[0:DH, :])
                    nc.vector.tensor_copy(kTz[DH:P, 1, sl], ps[DH:P, :])
                    ps = pj_ps.tile([P, 512], F32, tag="qk_ps")
                    for c in range(NCC):
                        nc.tensor.matmul(
                            ps, wq_t[:, c, :], xt[:, c, sl],
                            start=(c == 0), stop=(c == NCC - 1),
                        )
                    if j % 2 == 0:
                        nc.scalar.copy(qT[:, sl], ps)
                    else:
                        nc.vector.tensor_copy(qT[:, sl], ps)

                # V: per 128-token tile, [tokens, 128] = xT-chunk.T @ wv
                nc.gpsimd.memset(v_store, 0.0)
                ones_t = small.tile([P, NKT], F32, tag="ones")
                nc.vector.memset(ones_t, 1.0)
                for h in range(HEADS_PER_CORE):
                    nc.vector.tensor_copy(v_store[:, :, h, DH], ones_t)
                for r in range(NKT):
                    ps = pj_ps.tile([P, P], F32, tag="v_ps")
                    for c in range(NCC):
                        nc.tensor.matmul(
                            ps, xt[:, c, r * P:(r + 1) * P], wv_t[:, c, :],
                            start=(c == 0), stop=(c == NCC - 1),
                        )
                    nc.vector.tensor_copy(v_store[:, r, 0, 0:DH], ps[:, 0:DH])
                    nc.scalar.copy(v_store[:, r, 1, 0:DH], ps[:, DH:2 * DH])

            # ---- attention ----
            s_ps_cm = tc.tile_pool(name="s_ps", bufs=3, space="PSUM")
            a_ps_cm = tc.tile_pool(name="a_ps", bufs=2, space="PSUM")
            s_ps = s_ps_cm.__enter__()
            a_ps = a_ps_cm.__enter__()
            for qc in range(NQC):
                qsl = slice(qc * QCHUNK, (qc + 1) * QCHUNK)
                att = [a_ps.tile([P, QCHUNK], F32, tag="att", name=f"att{_h}")
                       for _h in range(HEADS_PER_CORE)]
                pend = []  # (kt, pt) waiting for their AV matmuls
                for kt in range(NKT):
                    sps = s_ps.tile([P, 2 * QCHUNK], F32, tag="spsum")
                    for h in range(HEADS_PER_CORE):
                        nc.tensor.matmul(
                            sps[:, h * QCHUNK:(h + 1) * QCHUNK],
                            kTz[:, h, kt * P:(kt + 1) * P],
                            qT[:, qsl],
                            start=True, stop=True,
                        )
                    pt = ptp.tile([P, 2 * QCHUNK], BF16, tag="pt")
                    if kt % 2 == 1:
                        nc.vector.tensor_scalar(
                            pt.bitcast(I16), sps, SCH_A, SCH_B, MULT, ADD)
                    else:
                        nc.scalar.activation(pt, sps, EXP)
                    pend.append((kt, pt))
                    if len(pend) > 1:
                        _emit_av(nc, att, v_store, *pend.pop(0))
                for p in pend:
                    _emit_av(nc, att, v_store, *p)
                # drain unnormalized attention + denominator row
                nc.scalar.copy(attn[0][:, qsl], att[0][0:DH + 1, :])
                nc.vector.tensor_copy(attn[1][:, qsl], att[1][0:DH + 1, :])
            a_ps_cm.__exit__(None, None, None)
            s_ps_cm.__exit__(None, None, None)

            # ---- output projection (per head, unnormalized) ----
            # all 8 PSUM banks are free here: deep ring keeps the PE dense
            for h in range(HEADS_PER_CORE):
                nc.sync.dma_start(den01[h:h + 1, :], attn[h][DH:DH + 1, :])
            with tc.tile_pool(name="o_ps", bufs=8, space="PSUM") as o_ps:
                osbs = [None, None]
                for qt in range(L // P):
                    for h, out_h in ((0, out0), (1, out1)):
                        ps = o_ps.tile([P, C], F32, tag="o_ps")
                        nc.tensor.matmul(
                            ps, attn[h][0:DH, qt * P:(qt + 1) * P],
                            wo_th[h], start=True, stop=True)
                        if qt % 4 == 0:
                            osbs[h] = outp.tile([P, 4, C], BF16, tag="osb",
                                                name=f"osb{h}")
                        if (2 * qt + h) % 2 == 0:
                            nc.scalar.copy(osbs[h][:, qt % 4, :], ps)
                        else:
                            nc.vector.tensor_copy(osbs[h][:, qt % 4, :], ps)
                        if qt % 4 == 3:
                            nc.sync.dma_start(
                                out_h[:, qt - 3:qt + 1, :], osbs[h])

    nc.compile()
    return nc


def _get_nc(reps=1):
    key = f"nc{reps}"
    if key not in _cached:
        _cached[key] = _build(reps)
    return _cached[key]


def _build_in_maps(inputs):
    x = np.asarray(inputs["x"], dtype=np.float32)
    Wq = np.asarray(inputs["Wq"], dtype=np.float32)
    Wk = np.asarray(inputs["Wk"], dtype=np.float32)
    Wv = np.asarray(inputs["Wv"], dtype=np.float32)
    Wo = np.asarray(inputs["Wo"], dtype=np.float32)

    scale = np.float32(1.0 / np.sqrt(DH))
    in_maps = []
    for core in range(NCORES):
        b = core // CORES_PER_BATCH
        j = core % CORES_PER_BATCH
        csl = slice(j * P, (j + 1) * P)
        bf = ml_dtypes.bfloat16
        in_maps.append({
            "xT": np.ascontiguousarray(
                x[b].T.reshape(NCC, P, L).transpose(1, 0, 2).astype(bf)),
            "wq": np.ascontiguousarray((Wq[:, csl] * scale).astype(bf)
                                       .reshape(NCC, P, P).transpose(1, 0, 2)),
            "wk": np.ascontiguousarray(Wk[:, csl].astype(bf)
                                       .reshape(NCC, P, P).transpose(1, 0, 2)),
            "wv": np.ascontiguousarray(Wv[:, csl].astype(bf)
                                       .reshape(NCC, P, P).transpose(1, 0, 2)),
            "wo": np.ascontiguousarray(Wo[csl, :].astype(bf)),
        })
    return in_maps


def kernel(x, Wq, Wk, Wv, Wo, bo):
    from concourse import bass_utils

    bo = np.asarray(bo, dtype=np.float32)
    in_maps = _build_in_maps(
        {"x": x, "Wq": Wq, "Wk": Wk, "Wv": Wv, "Wo": Wo})

    res = bass_utils.run_bass_kernel_spmd(
        _get_nc(), in_maps, core_ids=list(range(NCORES)))

    out = np.zeros((B, L, C), dtype=np.float32)
    for core in range(NCORES):
        r = res.results[core]
        den = np.asarray(r["den01"]).astype(np.float32)  # [2, L]
        b = core // CORES_PER_BATCH
        o0 = np.asarray(r["out0"]).astype(np.float32)
        o1 = np.asarray(r["out1"]).astype(np.float32)
        o0 = o0.transpose(1, 0, 2).reshape(L, C)
        o1 = o1.transpose(1, 0, 2).reshape(L, C)
        out[b] += o0 / den[0][:, None] + o1 / den[1][:, None]
    out += bo[None, None, :]
    return out
